# revision 12
# baseline (speedup 1.0000x reference)
"""Distributed Trainium2 kernel for nn_Criterion_35012573397697 (v2).

Proxy-NCA-style loss: mean_b[ d(x_b, p_{y_b}) + logsumexp_{c != y_b}(-d(x_b, p_c)) ]
with x = 3*l2norm(batch), p = 3*l2norm(proxies), d = squared euclidean.
d(x,p) = 18 - 2*s with s = x_hat . p_hat, so only dot products are needed.

v2 strategy (8 NeuronCores, classes sharded, 12500+300pad = 12800 per core):
  - Host permutes each core's proxy shard to partition-major [128, 100, 128]
    (pure indexing) so GpSimd SWDGE cast-DMAs load it straight from DRAM
    f32 into SBUF bf16 with big contiguous descriptors.
  - Norms per 8-tile chunk: square on GpSimd (bf16), reduce over d on DVE,
    3/|p| via the DVE rsqrt bit-trick (the only ScalarE act table used is
    the exp/ln one - sqrt would thrash ACT_TABLE_LOADs), per-tile scale on
    DVE in 4x mode. The XBAR dma-transpose (SP queue) then produces
    pT [d, t, c] in SBUF directly - no PE transposes, no PSUM->SBUF copies.
  - Matmul units of (bt, 2048 classes): 4 matmuls [128b, 512c] into a
    4-bank PSUM group. Exp is split: most units on ScalarE
    (exp(2s-18) with fused accum row-sum), a tunable subset on DVE via a
    Schraudolph bit-trick exp (f32->int16 bitcast-bf16) + tree row-sum.
  - Small AllGather of per-b partial sums; every core computes the same
    final scalar. pos-distance exact in f32 from host-gathered (indexing
    only) proxies[labels]; exp(-pos) and the zero-pad contribution are
    subtracted before the log.
"""

import math

import numpy as np
import ml_dtypes

import concourse.bass as bass
import concourse.bacc as bacc
import concourse.mybir as mybir
import concourse.tile as tile
from concourse.bass_utils import run_bass_kernel_spmd

N_CORES = 8
B = 512
D = 128
C = 100000
SH = 12800           # padded shard size per core
NT = SH // 128       # 100 c-tiles of 128
BT = B // 128        # 4 b-tiles
PAD_ROWS = N_CORES * SH - C   # 2400 zero rows in total
PAD_CORR = PAD_ROWS * math.exp(-18.0)

F32 = mybir.dt.float32
BF16 = mybir.dt.bfloat16
I16 = mybir.dt.int16
AX = mybir.AxisListType
OP = mybir.AluOpType
AF = mybir.ActivationFunctionType

# Schraudolph exp(2s-18) in bf16 bit space: i16 = s*SCH_A + SCH_B
LOG2E = 1.4426950408889634
SCH_A = 2.0 * LOG2E * 128.0
SCH_B = 16256.0 - 18.0 * LOG2E * 128.0 - 7.35

# norm-pipeline chunks (in c-tiles)
CHUNKS = [4] + [8] * 12
# DMA batches (in tiles) for the SWDGE cast loads
DMA_BATCHES = [4, 8, 16, 24, 24, 24]
# exp/matmul units: each covers (bt, UNIT_T c-tiles); last unit is smaller.
UNIT_T = 16
UNIT_RANGES = [(u * UNIT_T, min((u + 1) * UNIT_T, NT))
               for u in range((NT + UNIT_T - 1) // UNIT_T)]   # 7 ranges
N_UNITS = len(UNIT_RANGES) * BT                               # 28
# which unit-ranges use the DVE Schraudolph path (never the last: pads
# must go through exact ScalarE exp). Tunable.
DVE_RANGES = set()

_CACHE = {}


def _rsqrt_dve(nc, pool, dst, src, n, scale=1.0, newton=2):
    """dst = scale / sqrt(src) via bit trick + Newton steps (DVE only)."""
    I32 = mybir.dt.int32
    v = pool.tile([128, n], F32, tag="rsq_v")
    nc.vector.tensor_scalar(v[:], src, 1e-12, None, OP.max)
    src = v[:]
    h = pool.tile([128, n], I32, tag="rsq_h")
    nc.vector.tensor_scalar(h[:], src.bitcast(I32), 1, None,
                            OP.logical_shift_right)
    y0 = pool.tile([128, n], I32, tag="rsq_y0")
    nc.vector.tensor_scalar(y0[:], h[:], -1, 0x5F3759DF, OP.mult, OP.add)
    y0f = y0[:].bitcast(F32)
    t = pool.tile([128, n], F32, tag="rsq_t")
    if newton == 1:
        nc.vector.tensor_tensor(t[:], y0f, y0f, OP.mult)
        nc.vector.tensor_tensor(t[:], t[:], src, OP.mult)
        nc.vector.tensor_scalar(t[:], t[:], -0.5 * scale, 1.5 * scale,
                                OP.mult, OP.add)
        nc.vector.tensor_tensor(dst, y0f, t[:], OP.mult)
        return
    y1 = pool.tile([128, n], F32, tag="rsq_y1")
    nc.vector.tensor_tensor(t[:], y0f, y0f, OP.mult)
    nc.vector.tensor_tensor(t[:], t[:], src, OP.mult)
    nc.vector.tensor_scalar(t[:], t[:], -0.5, 1.5, OP.mult, OP.add)
    nc.vector.tensor_tensor(y1[:], y0f, t[:], OP.mult)
    nc.vector.tensor_tensor(t[:], y1[:], y1[:], OP.mult)
    nc.vector.tensor_tensor(t[:], t[:], src, OP.mult)
    nc.vector.tensor_scalar(t[:], t[:], -0.5 * scale, 1.5 * scale,
                            OP.mult, OP.add)
    nc.vector.tensor_tensor(dst, y1[:], t[:], OP.mult)


def build_graph():
    nc = bacc.Bacc("TRN2", target_bir_lowering=False, debug=False,
                   num_devices=N_CORES)
    p_ext = nc.dram_tensor("pshard", [128, NT * D], F32,
                           kind="ExternalInput").ap()
    b_ext = nc.dram_tensor("batch", [B, D], F32, kind="ExternalInput").ap()
    sel_ext = nc.dram_tensor("psel", [B, D], F32, kind="ExternalInput").ap()
    id_ext = nc.dram_tensor("ident", [128, 128], BF16,
                            kind="ExternalInput").ap()
    out_ext = nc.dram_tensor("out", [1, 1], F32, kind="ExternalOutput").ap()

    psrc = p_ext.rearrange("p (t d) -> p t d", t=NT)

    with tile.TileContext(nc) as tc:
        with tc.tile_pool(name="dram", bufs=1, space="DRAM") as dram, \
             tc.tile_pool(name="big", bufs=1) as bigp, \
             tc.tile_pool(name="sb", bufs=2) as pool, \
             tc.tile_pool(name="sq", bufs=3) as sqp, \
             tc.tile_pool(name="ps", bufs=2, space="PSUM") as psp:

            # ---------- persistent SBUF ----------
            pbf = bigp.tile([128, NT, D], BF16, tag="pbf")    # [c%128, t, d]
            pT = bigp.tile([128, NT, D], BF16, tag="pT")      # [d, t, c%128]
            n2 = bigp.tile([128, NT], F32, tag="n2")
            kk = bigp.tile([128, NT], F32, tag="kk")          # 3/|p|
            partials = bigp.tile([128, N_UNITS], F32, tag="partials")
            bias18 = bigp.tile([128, 1], F32, tag="bias18")
            nc.vector.memset(bias18[:], -18.0)
            xb = bigp.tile([128, BT, 128], F32, tag="xb")
            selb = bigp.tile([128, BT, 128], F32, tag="selb")
            ident = bigp.tile([128, 128], BF16, tag="ident")

            # ---------- input DMAs ----------
            nc.sync.dma_start(xb[:], b_ext.rearrange("(t p) d -> p t d", p=128))
            nc.sync.dma_start(selb[:],
                              sel_ext.rearrange("(t p) d -> p t d", p=128))
            nc.sync.dma_start(ident[:], id_ext[:])

            # cast-DMA loads of the proxy shard (f32 DRAM -> bf16 SBUF).
            # Only the first two batches are issued up front; the rest are
            # interleaved into the chunk loop so SWDGE descriptor
            # generation on GpSimd doesn't delay the first squares.
            dma_bounds = []
            clo = 0
            for ck in DMA_BATCHES:
                dma_bounds.append((clo, clo + ck))
                clo += ck

            def issue_cast_dma(i):
                lo_, hi_ = dma_bounds[i]
                nc.gpsimd.dma_start(pbf[:, lo_:hi_, :], psrc[:, lo_:hi_, :])

            issue_cast_dma(0)
            issue_cast_dma(1)

            # early tiny AllGather to absorb first-collective warm-up;
            # folded in as exact zero at the end.
            dag_in = dram.tile([1, 16], F32)
            dag_out = dram.tile([N_CORES, 16], F32)
            z16 = bigp.tile([1, 16], F32, tag="z16")
            nc.vector.memset(z16[:], 0.0)
            dagj = bigp.tile([1, 1], F32, tag="dagj")
            nc.sync.dma_start(dag_in[:], z16[:])
            nc.gpsimd.collective_compute(
                "AllGather", OP.bypass,
                replica_groups=[list(range(N_CORES))],
                ins=[dag_in.opt()], outs=[dag_out.opt()],
            )
            # NOTE: dagj is read AFTER the chunk loop - reading it here
            # would park the SP queue on the collective semaphore and
            # stall every dma_start_transpose behind it.

            # ---------- x side (tiny, exact f32) ----------
            xn2 = bigp.tile([128, 2 * BT], F32, tag="xn2")
            sqx = pool.tile([128, BT, 128], F32, tag="sqscr")
            nc.vector.tensor_tensor(sqx[:], xb[:], xb[:], OP.mult)
            nc.vector.tensor_reduce(xn2[:, 0:BT], sqx[:], axis=AX.X, op=OP.add)
            sqs = pool.tile([128, BT, 128], F32, tag="sqscr")
            nc.vector.tensor_tensor(sqs[:], selb[:], selb[:], OP.mult)
            nc.vector.tensor_reduce(xn2[:, BT:2 * BT], sqs[:], axis=AX.X,
                                    op=OP.add)
            rn = bigp.tile([128, 2 * BT], F32, tag="rn")   # 1/sqrt(xn2)
            _rsqrt_dve(nc, pool, rn[:], xn2[:], 2 * BT)

            posdot = bigp.tile([128, BT], F32, tag="posdot")
            sqd = pool.tile([128, BT, 128], F32, tag="sqscr")
            nc.vector.tensor_tensor(sqd[:], xb[:], selb[:], OP.mult)
            nc.vector.tensor_reduce(posdot[:], sqd[:], axis=AX.X, op=OP.add)
            posd = bigp.tile([128, BT], F32, tag="posd")
            tmp4 = pool.tile([128, BT], F32, tag="smallscr")
            nc.vector.tensor_tensor(tmp4[:], posdot[:], rn[:, 0:BT], OP.mult)
            nc.vector.tensor_tensor(tmp4[:], tmp4[:], rn[:, BT:2 * BT],
                                    OP.mult)
            nc.vector.tensor_scalar(posd[:], tmp4[:], -18.0, 18.0, OP.mult,
                                    OP.add)

            xscale3 = bigp.tile([128, BT], F32, tag="xscale3")
            nc.vector.tensor_scalar_mul(xscale3[:], rn[:, 0:BT], 3.0)
            xhat = bigp.tile([128, BT, 128], BF16, tag="xhat")
            for t in range(BT):
                nc.vector.tensor_scalar_mul(xhat[:, t, :], xb[:, t, :],
                                            xscale3[:, t:t + 1])
            xT = bigp.tile([128, BT, 128], BF16, tag="xT")
            xps = psp.tile([128, BT * 128], BF16, tag="ps")
            for t in range(BT):
                nc.tensor.transpose(xps[:, t * 128:(t + 1) * 128],
                                    xhat[:, t, :], ident[:])
            nc.vector.tensor_copy(
                xT[:], xps[:, 0:BT * 128].rearrange("p (t d) -> p t d", t=BT))

            # ---------- per-chunk norm pipeline + units ----------
            def issue_unit(uidx, bt, lo, hi):
                w = hi - lo                      # tiles (<= UNIT_T)
                nclass = w * 128
                sp = psp.tile([128, 2048], F32, tag="ps")
                pTf = pT[:].rearrange("p t c -> p (t c)")
                for j in range(0, w, 4):
                    j2 = min(j + 4, w)
                    nc.tensor.matmul(
                        sp[:, j * 128:j2 * 128],
                        xT[:, bt, :],
                        pTf[:, (lo + j) * 128:(lo + j2) * 128],
                        start=True, stop=True)
                col = partials[:, uidx:uidx + 1]
                rng_idx = lo // UNIT_T
                if rng_idx in DVE_RANGES:
                    sch = pool.tile([128, 2048], I16, tag="sch")
                    nc.vector.tensor_scalar(sch[:, 0:nclass], sp[:, 0:nclass],
                                            SCH_A, SCH_B, OP.mult, OP.add)
                    schb = sch[:].bitcast(BF16)
                    h = nclass // 2
                    t1 = pool.tile([128, 1024], BF16, tag="tree1")
                    nc.vector.tensor_tensor(t1[:, 0:h], schb[:, 0:h],
                                            schb[:, h:nclass], OP.add)
                    q = h // 2
                    t2 = pool.tile([128, 512], BF16, tag="tree2")
                    nc.vector.tensor_tensor(t2[:, 0:q], t1[:, 0:q],
                                            t1[:, q:h], OP.add)
                    nc.vector.tensor_reduce(col, t2[:, 0:q], axis=AX.X,
                                            op=OP.add)
                else:
                    ej = pool.tile([128, 2048], BF16, tag="ejunk")
                    nc.scalar.activation(
                        ej[:, 0:nclass], sp[:, 0:nclass], AF.Exp,
                        bias=bias18[:, 0:1], scale=2.0, accum_out=col)

            unit_issued = 0
            next_dma = 2
            chunk_bounds = []
            clo = 0
            for ck in CHUNKS:
                chunk_bounds.append((clo, clo + ck))
                clo += ck

            # rsqrt over pairs of chunks to amortize the 10-instr bit trick
            rsq_pairs = {}   # chunk index -> (lo, hi) handled at that index
            ci = 0
            while ci < len(CHUNKS):
                if ci + 1 < len(CHUNKS):
                    rsq_pairs[ci + 1] = (chunk_bounds[ci][0],
                                         chunk_bounds[ci + 1][1])
                    ci += 2
                else:
                    rsq_pairs[ci] = chunk_bounds[ci]
                    ci += 1

            for ci, (lo, hi) in enumerate(chunk_bounds):
                ck = hi - lo
                # keep SWDGE generation ~1 batch ahead of consumption
                if next_dma < len(dma_bounds) and \
                        dma_bounds[next_dma - 1][1] <= hi + 8:
                    issue_cast_dma(next_dma)
                    next_dma += 1
                psq = sqp.tile([128, 8, D], BF16, tag="psq")
                nc.gpsimd.tensor_tensor(psq[:, 0:ck, :], pbf[:, lo:hi, :],
                                        pbf[:, lo:hi, :], OP.mult)
                nc.vector.tensor_reduce(n2[:, lo:hi], psq[:, 0:ck, :],
                                        axis=AX.X, op=OP.add)
                if ci in rsq_pairs:
                    rlo, rhi = rsq_pairs[ci]
                    _rsqrt_dve(nc, pool, kk[:, rlo:rhi], n2[:, rlo:rhi],
                               rhi - rlo, scale=3.0)
                    # scale + transpose for everything the rsqrt covered
                    pscl = sqp.tile([128, 16, D], BF16, tag="pscl")
                    nc.vector.tensor_tensor(
                        pscl[:, 0:rhi - rlo, :], pbf[:, rlo:rhi, :],
                        kk[:, rlo:rhi, None].to_broadcast(
                            (128, rhi - rlo, D)),
                        OP.mult)
                    nc.sync.dma_start_transpose(
                        pT[:, rlo:rhi, :],
                        pscl[:, 0:rhi - rlo, :].rearrange("p t d -> p (t d)"))
                    # issue any unit fully covered by transposed tiles
                    while unit_issued < len(UNIT_RANGES) and \
                            UNIT_RANGES[unit_issued][1] <= rhi:
                        ulo, uhi = UNIT_RANGES[unit_issued]
                        for bt in range(BT):
                            issue_unit(unit_issued * BT + bt, bt, ulo, uhi)
                        unit_issued += 1

            # deferred dummy-AG result read (SP queue is past all transposes)
            nc.sync.dma_start(dagj[:], dag_out[0:1, 0:1])

            # ---------- local partial sums ----------
            s_loc = bigp.tile([128, BT], F32, tag="s_loc")
            nc.vector.tensor_reduce(
                s_loc[:],
                partials[:].rearrange("p (u t) -> p t u", t=BT),
                axis=AX.X, op=OP.add)

            # ---------- AllGather + final ----------
            ag_in = dram.tile([128, BT], F32)
            ag_out = dram.tile([128 * N_CORES, BT], F32)
            nc.sync.dma_start(ag_in[:], s_loc[:])
            nc.gpsimd.collective_compute(
                "AllGather", OP.bypass,
                replica_groups=[list(range(N_CORES))],
                ins=[ag_in.opt()], outs=[ag_out.opt()],
            )
            gath = bigp.tile([128, BT, N_CORES], F32, tag="gath")
            nc.sync.dma_start(gath[:],
                              ag_out.rearrange("(r p) f -> p f r", p=128))
            s_tot = bigp.tile([128, BT], F32, tag="s_tot")
            nc.vector.tensor_reduce(s_tot[:], gath[:], axis=AX.X, op=OP.add)

            npos = pool.tile([128, BT], F32, tag="fin")
            nc.scalar.activation(npos[:], posd[:], AF.Exp, scale=-1.0)
            s1 = pool.tile([128, BT], F32, tag="fin")
            nc.vector.tensor_scalar(s1[:], s_tot[:], -float(PAD_CORR),
                                    None, OP.add)
            nc.vector.tensor_tensor(s1[:], s1[:], npos[:], OP.subtract)
            lse = pool.tile([128, BT], F32, tag="fin")
            nc.scalar.activation(lse[:], s1[:], AF.Ln)
            perb = pool.tile([128, BT], F32, tag="fin")
            nc.vector.tensor_tensor(perb[:], posd[:], lse[:], OP.add)
            csum = pool.tile([128, 1], F32, tag="fin")
            nc.vector.tensor_reduce(csum[:], perb[:], axis=AX.X, op=OP.add)
            nc.vector.tensor_tensor(csum[0:1, 0:1], csum[0:1, 0:1],
                                    dagj[:], OP.add)
            ones = pool.tile([128, 1], F32, tag="fin")
            nc.vector.memset(ones[:], 1.0)
            lps = psp.tile([128, 2048], F32, tag="ps")
            nc.tensor.matmul(lps[0:1, 0:1], ones[:], csum[:], start=True,
                             stop=True)
            res = pool.tile([1, 1], F32, tag="fin")
            nc.scalar.activation(res[:], lps[0:1, 0:1], AF.Copy, scale=1.0 / B)
            nc.sync.dma_start(out_ext[:], res[:])

    nc.compile()
    return nc


def make_in_maps(batch, labels, proxies):
    batch = np.ascontiguousarray(batch, dtype=np.float32)
    labels = np.asarray(labels).astype(np.int64)
    proxies = np.ascontiguousarray(proxies, dtype=np.float32)
    psel = np.ascontiguousarray(proxies[labels])        # indexing only
    ident = np.eye(128, dtype=np.float32).astype(ml_dtypes.bfloat16)
    ppad = np.zeros((N_CORES * SH, D), dtype=np.float32)
    ppad[:C] = proxies
    in_maps = []
    for i in range(N_CORES):
        shard = ppad[i * SH:(i + 1) * SH]
        # partition-major permutation (pure indexing): [128, NT*D]
        perm = shard.reshape(NT, 128, D).transpose(1, 0, 2).reshape(128,
                                                                    NT * D)
        in_maps.append({
            "pshard": np.ascontiguousarray(perm),
            "batch": batch,
            "psel": psel,
            "ident": ident,
        })
    return in_maps


def _get_nc():
    if "nc" not in _CACHE:
        _CACHE["nc"] = build_graph()
    return _CACHE["nc"]


def kernel(batch, labels, proxies):
    nc = _get_nc()
    in_maps = make_in_maps(batch, labels, proxies)
    try:
        res = run_bass_kernel_spmd(nc, in_maps, core_ids=list(range(N_CORES)))
    except Exception:
        # transient device hiccup: retry once
        res = run_bass_kernel_spmd(nc, in_maps, core_ids=list(range(N_CORES)))
    return np.float32(res.results[0]["out"][0, 0])


if __name__ == "__main__":
    rng = np.random.default_rng(0)
    batch = rng.standard_normal((B, D)).astype(np.float32)
    labels = rng.integers(0, C, B).astype(np.int64)
    proxies = (rng.standard_normal((C, D)).astype(np.float32) / 8)
    out = kernel(batch=batch, labels=labels, proxies=proxies)
    print("loss:", out)


# revision 14
# speedup vs baseline: 1.1520x; 1.1520x over previous
"""Distributed Trainium2 kernel for nn_Criterion_35012573397697 (v2).

Proxy-NCA-style loss: mean_b[ d(x_b, p_{y_b}) + logsumexp_{c != y_b}(-d(x_b, p_c)) ]
with x = 3*l2norm(batch), p = 3*l2norm(proxies), d = squared euclidean.
d(x,p) = 18 - 2*s with s = x_hat . p_hat, so only dot products are needed.

v2 strategy (8 NeuronCores, classes sharded, 12500+300pad = 12800 per core):
  - Host permutes each core's proxy shard to partition-major [128, 100, 128]
    (pure indexing) so GpSimd SWDGE cast-DMAs load it straight from DRAM
    f32 into SBUF bf16 with big contiguous descriptors.
  - Norms per 8-tile chunk: square on GpSimd (bf16), reduce over d on DVE,
    3/|p| via the DVE rsqrt bit-trick (the only ScalarE act table used is
    the exp/ln one - sqrt would thrash ACT_TABLE_LOADs), per-tile scale on
    DVE in 4x mode. The XBAR dma-transpose (SP queue) then produces
    pT [d, t, c] in SBUF directly - no PE transposes, no PSUM->SBUF copies.
  - Matmul units of (bt, 2048 classes): 4 matmuls [128b, 512c] into a
    4-bank PSUM group. Exp is split: most units on ScalarE
    (exp(2s-18) with fused accum row-sum), a tunable subset on DVE via a
    Schraudolph bit-trick exp (f32->int16 bitcast-bf16) + tree row-sum.
  - Small AllGather of per-b partial sums; every core computes the same
    final scalar. pos-distance exact in f32 from host-gathered (indexing
    only) proxies[labels]; exp(-pos) and the zero-pad contribution are
    subtracted before the log.
"""

import math

import numpy as np
import ml_dtypes

import concourse.bass as bass
import concourse.bacc as bacc
import concourse.mybir as mybir
import concourse.tile as tile
from concourse.bass_utils import run_bass_kernel_spmd

N_CORES = 8
B = 512
D = 128
C = 100000
SH = 12800           # padded shard size per core
NT = SH // 128       # 100 c-tiles of 128
BT = B // 128        # 4 b-tiles
PAD_ROWS = N_CORES * SH - C   # 2400 zero rows in total
PAD_CORR = PAD_ROWS * math.exp(-18.0)

F32 = mybir.dt.float32
BF16 = mybir.dt.bfloat16
I16 = mybir.dt.int16
AX = mybir.AxisListType
OP = mybir.AluOpType
AF = mybir.ActivationFunctionType

# Schraudolph exp(2s-18) in bf16 bit space: i16 = s*SCH_A + SCH_B
LOG2E = 1.4426950408889634
SCH_A = 2.0 * LOG2E * 128.0
SCH_B = 16256.0 - 18.0 * LOG2E * 128.0 - 7.35

# norm-pipeline chunks (in c-tiles)
CHUNKS = [4] + [8] * 12
# DMA batches (in tiles) for the SWDGE cast loads
DMA_BATCHES = [4, 8, 16, 24, 24, 24]
# exp/matmul units: each covers (bt, UNIT_T c-tiles); last unit is smaller.
UNIT_T = 16
UNIT_RANGES = [(u * UNIT_T, min((u + 1) * UNIT_T, NT))
               for u in range((NT + UNIT_T - 1) // UNIT_T)]   # 7 ranges
N_UNITS = len(UNIT_RANGES) * BT                               # 28
# which unit-ranges use the DVE Schraudolph path (never the last: pads
# must go through exact ScalarE exp). Tunable.
DVE_RANGES = set()

_CACHE = {}


def _rsqrt_dve(nc, pool, dst, src, n, scale=1.0, newton=2):
    """dst = scale / sqrt(src) via bit trick + Newton steps (DVE only)."""
    I32 = mybir.dt.int32
    v = pool.tile([128, n], F32, tag="rsq_v")
    nc.vector.tensor_scalar(v[:], src, 1e-12, None, OP.max)
    src = v[:]
    h = pool.tile([128, n], I32, tag="rsq_h")
    nc.vector.tensor_scalar(h[:], src.bitcast(I32), 1, None,
                            OP.logical_shift_right)
    y0 = pool.tile([128, n], I32, tag="rsq_y0")
    nc.vector.tensor_scalar(y0[:], h[:], -1, 0x5F3759DF, OP.mult, OP.add)
    y0f = y0[:].bitcast(F32)
    t = pool.tile([128, n], F32, tag="rsq_t")
    if newton == 1:
        nc.vector.tensor_tensor(t[:], y0f, y0f, OP.mult)
        nc.vector.tensor_tensor(t[:], t[:], src, OP.mult)
        nc.vector.tensor_scalar(t[:], t[:], -0.5 * scale, 1.5 * scale,
                                OP.mult, OP.add)
        nc.vector.tensor_tensor(dst, y0f, t[:], OP.mult)
        return
    y1 = pool.tile([128, n], F32, tag="rsq_y1")
    nc.vector.tensor_tensor(t[:], y0f, y0f, OP.mult)
    nc.vector.tensor_tensor(t[:], t[:], src, OP.mult)
    nc.vector.tensor_scalar(t[:], t[:], -0.5, 1.5, OP.mult, OP.add)
    nc.vector.tensor_tensor(y1[:], y0f, t[:], OP.mult)
    nc.vector.tensor_tensor(t[:], y1[:], y1[:], OP.mult)
    nc.vector.tensor_tensor(t[:], t[:], src, OP.mult)
    nc.vector.tensor_scalar(t[:], t[:], -0.5 * scale, 1.5 * scale,
                            OP.mult, OP.add)
    nc.vector.tensor_tensor(dst, y1[:], t[:], OP.mult)


def build_graph():
    nc = bacc.Bacc("TRN2", target_bir_lowering=False, debug=False,
                   num_devices=N_CORES)
    p_ext = nc.dram_tensor("pshard", [128, NT * D], F32,
                           kind="ExternalInput").ap()
    b_ext = nc.dram_tensor("batch", [B, D], F32, kind="ExternalInput").ap()
    sel_ext = nc.dram_tensor("psel", [B, D], F32, kind="ExternalInput").ap()
    id_ext = nc.dram_tensor("ident", [128, 128], BF16,
                            kind="ExternalInput").ap()
    out_ext = nc.dram_tensor("out", [1, 1], F32, kind="ExternalOutput").ap()

    psrc = p_ext.rearrange("p (t d) -> p t d", t=NT)

    with tile.TileContext(nc) as tc:
        with tc.tile_pool(name="dram", bufs=1, space="DRAM") as dram, \
             tc.tile_pool(name="big", bufs=1) as bigp, \
             tc.tile_pool(name="sb", bufs=2) as pool, \
             tc.tile_pool(name="sq", bufs=3) as sqp, \
             tc.tile_pool(name="ps", bufs=2, space="PSUM") as psp:

            # ---------- persistent SBUF ----------
            pbf = bigp.tile([128, NT, D], BF16, tag="pbf")    # [c%128, t, d]
            pT = bigp.tile([128, NT, D], BF16, tag="pT")      # [d, t, c%128]
            n2 = bigp.tile([128, NT], F32, tag="n2")
            kk = bigp.tile([128, NT], F32, tag="kk")          # 3/|p|
            partials = bigp.tile([128, N_UNITS], F32, tag="partials")
            bias18 = bigp.tile([128, 1], F32, tag="bias18")
            nc.vector.memset(bias18[:], -18.0)
            xb = bigp.tile([128, BT, 128], F32, tag="xb")
            selb = bigp.tile([128, BT, 128], F32, tag="selb")
            ident = bigp.tile([128, 128], BF16, tag="ident")

            # ---------- input DMAs ----------
            nc.sync.dma_start(xb[:], b_ext.rearrange("(t p) d -> p t d", p=128))
            nc.sync.dma_start(selb[:],
                              sel_ext.rearrange("(t p) d -> p t d", p=128))
            nc.sync.dma_start(ident[:], id_ext[:])

            # cast-DMA loads of the proxy shard (f32 DRAM -> bf16 SBUF).
            # Only the first two batches are issued up front; the rest are
            # interleaved into the chunk loop so SWDGE descriptor
            # generation on GpSimd doesn't delay the first squares.
            dma_bounds = []
            clo = 0
            for ck in DMA_BATCHES:
                dma_bounds.append((clo, clo + ck))
                clo += ck

            def issue_cast_dma(i):
                lo_, hi_ = dma_bounds[i]
                nc.gpsimd.dma_start(pbf[:, lo_:hi_, :], psrc[:, lo_:hi_, :])

            issue_cast_dma(0)
            issue_cast_dma(1)

            # (dummy warm-up AllGather removed: its completion semaphore
            # serialized the dma_start_transpose stream behind the
            # cross-core start skew.)
            dagj = bigp.tile([1, 1], F32, tag="dagj")
            nc.vector.memset(dagj[:], 0.0)

            # ---------- x side (tiny, exact f32) ----------
            xn2 = bigp.tile([128, 2 * BT], F32, tag="xn2")
            sqx = pool.tile([128, BT, 128], F32, tag="sqscr")
            nc.vector.tensor_tensor(sqx[:], xb[:], xb[:], OP.mult)
            nc.vector.tensor_reduce(xn2[:, 0:BT], sqx[:], axis=AX.X, op=OP.add)
            sqs = pool.tile([128, BT, 128], F32, tag="sqscr")
            nc.vector.tensor_tensor(sqs[:], selb[:], selb[:], OP.mult)
            nc.vector.tensor_reduce(xn2[:, BT:2 * BT], sqs[:], axis=AX.X,
                                    op=OP.add)
            rn = bigp.tile([128, 2 * BT], F32, tag="rn")   # 1/sqrt(xn2)
            _rsqrt_dve(nc, pool, rn[:], xn2[:], 2 * BT)

            posdot = bigp.tile([128, BT], F32, tag="posdot")
            sqd = pool.tile([128, BT, 128], F32, tag="sqscr")
            nc.vector.tensor_tensor(sqd[:], xb[:], selb[:], OP.mult)
            nc.vector.tensor_reduce(posdot[:], sqd[:], axis=AX.X, op=OP.add)
            posd = bigp.tile([128, BT], F32, tag="posd")
            tmp4 = pool.tile([128, BT], F32, tag="smallscr")
            nc.vector.tensor_tensor(tmp4[:], posdot[:], rn[:, 0:BT], OP.mult)
            nc.vector.tensor_tensor(tmp4[:], tmp4[:], rn[:, BT:2 * BT],
                                    OP.mult)
            nc.vector.tensor_scalar(posd[:], tmp4[:], -18.0, 18.0, OP.mult,
                                    OP.add)

            xscale3 = bigp.tile([128, BT], F32, tag="xscale3")
            nc.vector.tensor_scalar_mul(xscale3[:], rn[:, 0:BT], 3.0)
            xhat = bigp.tile([128, BT, 128], BF16, tag="xhat")
            for t in range(BT):
                nc.vector.tensor_scalar_mul(xhat[:, t, :], xb[:, t, :],
                                            xscale3[:, t:t + 1])
            xT = bigp.tile([128, BT, 128], BF16, tag="xT")
            xps = psp.tile([128, BT * 128], BF16, tag="ps")
            for t in range(BT):
                nc.tensor.transpose(xps[:, t * 128:(t + 1) * 128],
                                    xhat[:, t, :], ident[:])
            nc.vector.tensor_copy(
                xT[:], xps[:, 0:BT * 128].rearrange("p (t d) -> p t d", t=BT))

            # ---------- per-chunk norm pipeline + units ----------
            def issue_unit(uidx, bt, lo, hi):
                w = hi - lo                      # tiles (<= UNIT_T)
                nclass = w * 128
                sp = psp.tile([128, 2048], F32, tag="ps")
                pTf = pT[:].rearrange("p t c -> p (t c)")
                for j in range(0, w, 4):
                    j2 = min(j + 4, w)
                    nc.tensor.matmul(
                        sp[:, j * 128:j2 * 128],
                        xT[:, bt, :],
                        pTf[:, (lo + j) * 128:(lo + j2) * 128],
                        start=True, stop=True)
                col = partials[:, uidx:uidx + 1]
                rng_idx = lo // UNIT_T
                if rng_idx in DVE_RANGES:
                    sch = pool.tile([128, 2048], I16, tag="sch")
                    nc.vector.tensor_scalar(sch[:, 0:nclass], sp[:, 0:nclass],
                                            SCH_A, SCH_B, OP.mult, OP.add)
                    schb = sch[:].bitcast(BF16)
                    h = nclass // 2
                    t1 = pool.tile([128, 1024], BF16, tag="tree1")
                    nc.vector.tensor_tensor(t1[:, 0:h], schb[:, 0:h],
                                            schb[:, h:nclass], OP.add)
                    q = h // 2
                    t2 = pool.tile([128, 512], BF16, tag="tree2")
                    nc.vector.tensor_tensor(t2[:, 0:q], t1[:, 0:q],
                                            t1[:, q:h], OP.add)
                    nc.vector.tensor_reduce(col, t2[:, 0:q], axis=AX.X,
                                            op=OP.add)
                else:
                    ej = pool.tile([128, 2048], BF16, tag="ejunk")
                    nc.scalar.activation(
                        ej[:, 0:nclass], sp[:, 0:nclass], AF.Exp,
                        bias=bias18[:, 0:1], scale=2.0, accum_out=col)

            unit_issued = 0
            next_dma = 2
            chunk_bounds = []
            clo = 0
            for ck in CHUNKS:
                chunk_bounds.append((clo, clo + ck))
                clo += ck

            # rsqrt over pairs of chunks to amortize the 10-instr bit trick
            rsq_pairs = {}   # chunk index -> (lo, hi) handled at that index
            ci = 0
            while ci < len(CHUNKS):
                if ci + 1 < len(CHUNKS):
                    rsq_pairs[ci + 1] = (chunk_bounds[ci][0],
                                         chunk_bounds[ci + 1][1])
                    ci += 2
                else:
                    rsq_pairs[ci] = chunk_bounds[ci]
                    ci += 1

            for ci, (lo, hi) in enumerate(chunk_bounds):
                ck = hi - lo
                # keep SWDGE generation ~1 batch ahead of consumption
                if next_dma < len(dma_bounds) and \
                        dma_bounds[next_dma - 1][1] <= hi + 8:
                    issue_cast_dma(next_dma)
                    next_dma += 1
                psq = sqp.tile([128, 8, D], BF16, tag="psq")
                nc.gpsimd.tensor_tensor(psq[:, 0:ck, :], pbf[:, lo:hi, :],
                                        pbf[:, lo:hi, :], OP.mult)
                nc.vector.tensor_reduce(n2[:, lo:hi], psq[:, 0:ck, :],
                                        axis=AX.X, op=OP.add)
                if ci in rsq_pairs:
                    rlo, rhi = rsq_pairs[ci]
                    _rsqrt_dve(nc, pool, kk[:, rlo:rhi], n2[:, rlo:rhi],
                               rhi - rlo, scale=3.0)
                    # scale + transpose for everything the rsqrt covered
                    pscl = sqp.tile([128, 16, D], BF16, tag="pscl")
                    nc.vector.tensor_tensor(
                        pscl[:, 0:rhi - rlo, :], pbf[:, rlo:rhi, :],
                        kk[:, rlo:rhi, None].to_broadcast(
                            (128, rhi - rlo, D)),
                        OP.mult)
                    nc.sync.dma_start_transpose(
                        pT[:, rlo:rhi, :],
                        pscl[:, 0:rhi - rlo, :].rearrange("p t d -> p (t d)"))
                    # issue any unit fully covered by transposed tiles
                    while unit_issued < len(UNIT_RANGES) and \
                            UNIT_RANGES[unit_issued][1] <= rhi:
                        ulo, uhi = UNIT_RANGES[unit_issued]
                        for bt in range(BT):
                            issue_unit(unit_issued * BT + bt, bt, ulo, uhi)
                        unit_issued += 1

            # ---------- local partial sums ----------
            s_loc = bigp.tile([128, BT], F32, tag="s_loc")
            nc.vector.tensor_reduce(
                s_loc[:],
                partials[:].rearrange("p (u t) -> p t u", t=BT),
                axis=AX.X, op=OP.add)

            # ---------- AllGather + final ----------
            ag_in = dram.tile([128, BT], F32)
            ag_out = dram.tile([128 * N_CORES, BT], F32)
            nc.sync.dma_start(ag_in[:], s_loc[:])
            nc.gpsimd.collective_compute(
                "AllGather", OP.bypass,
                replica_groups=[list(range(N_CORES))],
                ins=[ag_in.opt()], outs=[ag_out.opt()],
            )
            gath = bigp.tile([128, BT, N_CORES], F32, tag="gath")
            nc.sync.dma_start(gath[:],
                              ag_out.rearrange("(r p) f -> p f r", p=128))
            s_tot = bigp.tile([128, BT], F32, tag="s_tot")
            nc.vector.tensor_reduce(s_tot[:], gath[:], axis=AX.X, op=OP.add)

            npos = pool.tile([128, BT], F32, tag="fin")
            nc.scalar.activation(npos[:], posd[:], AF.Exp, scale=-1.0)
            s1 = pool.tile([128, BT], F32, tag="fin")
            nc.vector.tensor_scalar(s1[:], s_tot[:], -float(PAD_CORR),
                                    None, OP.add)
            nc.vector.tensor_tensor(s1[:], s1[:], npos[:], OP.subtract)
            lse = pool.tile([128, BT], F32, tag="fin")
            nc.scalar.activation(lse[:], s1[:], AF.Ln)
            perb = pool.tile([128, BT], F32, tag="fin")
            nc.vector.tensor_tensor(perb[:], posd[:], lse[:], OP.add)
            csum = pool.tile([128, 1], F32, tag="fin")
            nc.vector.tensor_reduce(csum[:], perb[:], axis=AX.X, op=OP.add)
            nc.vector.tensor_tensor(csum[0:1, 0:1], csum[0:1, 0:1],
                                    dagj[:], OP.add)
            ones = pool.tile([128, 1], F32, tag="fin")
            nc.vector.memset(ones[:], 1.0)
            lps = psp.tile([128, 2048], F32, tag="ps")
            nc.tensor.matmul(lps[0:1, 0:1], ones[:], csum[:], start=True,
                             stop=True)
            res = pool.tile([1, 1], F32, tag="fin")
            nc.scalar.activation(res[:], lps[0:1, 0:1], AF.Copy, scale=1.0 / B)
            nc.sync.dma_start(out_ext[:], res[:])

    nc.compile()
    return nc


def make_in_maps(batch, labels, proxies):
    batch = np.ascontiguousarray(batch, dtype=np.float32)
    labels = np.asarray(labels).astype(np.int64)
    proxies = np.ascontiguousarray(proxies, dtype=np.float32)
    psel = np.ascontiguousarray(proxies[labels])        # indexing only
    ident = np.eye(128, dtype=np.float32).astype(ml_dtypes.bfloat16)
    ppad = np.zeros((N_CORES * SH, D), dtype=np.float32)
    ppad[:C] = proxies
    in_maps = []
    for i in range(N_CORES):
        shard = ppad[i * SH:(i + 1) * SH]
        # partition-major permutation (pure indexing): [128, NT*D]
        perm = shard.reshape(NT, 128, D).transpose(1, 0, 2).reshape(128,
                                                                    NT * D)
        in_maps.append({
            "pshard": np.ascontiguousarray(perm),
            "batch": batch,
            "psel": psel,
            "ident": ident,
        })
    return in_maps


def _get_nc():
    if "nc" not in _CACHE:
        _CACHE["nc"] = build_graph()
    return _CACHE["nc"]


def kernel(batch, labels, proxies):
    nc = _get_nc()
    in_maps = make_in_maps(batch, labels, proxies)
    try:
        res = run_bass_kernel_spmd(nc, in_maps, core_ids=list(range(N_CORES)))
    except Exception:
        # transient device hiccup: retry once
        res = run_bass_kernel_spmd(nc, in_maps, core_ids=list(range(N_CORES)))
    return np.float32(res.results[0]["out"][0, 0])


if __name__ == "__main__":
    rng = np.random.default_rng(0)
    batch = rng.standard_normal((B, D)).astype(np.float32)
    labels = rng.integers(0, C, B).astype(np.int64)
    proxies = (rng.standard_normal((C, D)).astype(np.float32) / 8)
    out = kernel(batch=batch, labels=labels, proxies=proxies)
    print("loss:", out)
